# revision 1
# baseline (speedup 1.0000x reference)
"""Multi-head attention (B=4, S=2048, D=1024, H=16, causal) on 8 trn2 cores.

Sharding: core = b*2 + g  (b = batch 0..3, g = head-group 0..1, 8 heads each).
Per core, everything runs on-device in fp32r (full-rate PE):
  Q^T/K^T = (Wq/8)^T X^T etc. (d-major layout), V natural (s-major) with an
  appended ones-column per head (computes softmax denominators inside the AV
  matmul), transposed scores S^T = K_h Q_h^T per 128x512 block, causal via
  block skipping + additive -60 triangle mask (bf16 mask matmul accumulated
  into the scores PSUM), exp on ScalarE, O^T = V'^T P^T accumulated over key
  blocks, normalize by the ones-row reciprocal, final projection O @ Wo_g.
Host: input transpose + shard, and the g-pair partial-sum (row-parallel Wo
all-reduce) at gather time.
"""

import numpy as np

S = 2048
D = 1024
DL = 512          # local head dims per core (8 heads x 64)
HL = 8            # local heads
DK = 64
NB_K = D // 128   # contraction tiles for projections
NB_DB = DL // 128 # d-out blocks
NQ = S // 512     # q blocks
NB_S = S // 128   # s tiles / key blocks
MASK_VAL = -60.0

_NC = {}


def _build_nc(reps=1, phases="12BC", bench_internal=False):
    import concourse.bass as bass
    import concourse.mybir as mybir
    import concourse.tile as tile
    from concourse import bacc

    F32 = mybir.dt.float32
    F32R = mybir.dt.float32r
    BF16 = mybir.dt.bfloat16
    Exp = mybir.ActivationFunctionType.Exp

    nc = bacc.Bacc(None)

    big_kind = "Internal" if bench_internal else "ExternalInput"
    xqT = nc.dram_tensor("xqT", [D, S], F32R, kind=big_kind)
    xkT = nc.dram_tensor("xkT", [D, S], F32R, kind=big_kind)
    xvT = nc.dram_tensor("xvT", [D, S], F32R, kind=big_kind)
    wq = nc.dram_tensor("wq", [D, DL], F32R, kind=big_kind)
    wk = nc.dram_tensor("wk", [D, DL], F32R, kind=big_kind)
    wv = nc.dram_tensor("wv", [D, DL], F32R, kind=big_kind)
    wo = nc.dram_tensor("wo", [DL, D], F32R, kind=big_kind)
    bqs = nc.dram_tensor("bqs", [128, NB_DB], F32, kind="ExternalInput")
    bks = nc.dram_tensor("bks", [128, NB_DB], F32, kind="ExternalInput")
    bvrow = nc.dram_tensor("bvrow", [1, DL], F32R, kind="ExternalInput")
    borow = nc.dram_tensor("borow", [1, D], F32R, kind="ExternalInput")
    tri = nc.dram_tensor("tri", [128, 128], BF16, kind="ExternalInput")
    rect = nc.dram_tensor("rect", [128, 128], BF16, kind="ExternalInput")
    identb = nc.dram_tensor("identb", [128, 128], BF16, kind="ExternalInput")
    onesr = nc.dram_tensor("onesr", [1, 128], F32R, kind="ExternalInput")
    out_d = nc.dram_tensor("out", [S, D], F32, kind="ExternalOutput")

    with tile.TileContext(nc) as tc, nc.allow_low_precision(
            reason="fp32r rounding of matmul operands is intended"):
        with (
            tc.tile_pool(name="const", bufs=1) as cpool,
            tc.tile_pool(name="resident", bufs=1) as rpool,
        ):
            tri_sb = cpool.tile([128, 128], BF16, name="tri", tag="tri")
            rect_sb = cpool.tile([128, 128], BF16, name="rect", tag="rect")
            id_sb = cpool.tile([128, 128], BF16, name="identb", tag="identb")
            ones_sb = cpool.tile([1, 128], F32R, name="ones", tag="ones")
            bqs_sb = cpool.tile([128, NB_DB], F32, name="bqs", tag="bqs")
            bks_sb = cpool.tile([128, NB_DB], F32, name="bks", tag="bks")
            bv_sb = cpool.tile([1, DL], F32R, name="bv", tag="bv")
            bo_sb = cpool.tile([1, D], F32R, name="bo", tag="bo")
            for t, d in [(tri_sb, tri), (rect_sb, rect), (id_sb, identb),
                         (ones_sb, onesr), (bqs_sb, bqs), (bks_sb, bks),
                         (bv_sb, bvrow), (bo_sb, borow)]:
                nc.sync.dma_start(t[:], d[:])

            QT = [rpool.tile([128, S], F32R, name=f"QT{i}", tag=f"QT{i}") for i in range(NB_DB)]
            KT = [rpool.tile([128, S], F32R, name=f"KT{i}", tag=f"KT{i}") for i in range(NB_DB)]
            VT = [rpool.tile([128, HL, DK + 1], F32R, name=f"VT{i}", tag=f"VT{i}")
                  for i in range(NB_S)]
            OT = [rpool.tile([128, S], F32R, name=f"OT{i}", tag=f"OT{i}") for i in range(NB_DB)]

            for _rep in range(reps):
                if _rep == 0 or "1" in phases:
                    # ---- Phase A1: Q^T and K^T projections (d-major) ----
                    for xT, w, b_sb, dst, nm in [
                        (xqT, wq, bqs_sb, QT, "q"),
                        (xkT, wk, bks_sb, KT, "k"),
                    ]:
                        with (
                            tc.tile_pool(name=f"w{nm}", bufs=1) as wpool,
                            tc.tile_pool(name=f"x{nm}", bufs=3) as xpool,
                            tc.tile_pool(name=f"ps{nm}", bufs=2, space="PSUM") as pspool,
                        ):
                            w_sb = wpool.tile([128, NB_K, DL], F32R, name="w", tag="w")
                            nc.sync.dma_start(
                                w_sb[:], w.ap().rearrange("(kt p) n -> p kt n", p=128))
                            for s in range(NQ):
                                psums = [pspool.tile([128, 512], F32, name=f"pp{db}", tag=f"pp{db}")
                                         for db in range(NB_DB)]
                                for k in range(NB_K):
                                    xt = xpool.tile([128, 512], F32R, name="x", tag="x")
                                    nc.sync.dma_start(
                                        xt[:],
                                        xT[k * 128:(k + 1) * 128, s * 512:(s + 1) * 512])
                                    for db in range(NB_DB):
                                        nc.tensor.matmul(
                                            psums[db][:],
                                            w_sb[:, k, db * 128:(db + 1) * 128],
                                            xt[:],
                                            start=(k == 0), stop=(k == NB_K - 1))
                                for db in range(NB_DB):
                                    nc.vector.tensor_scalar_add(
                                        dst[db][:, s * 512:(s + 1) * 512],
                                        psums[db][:], b_sb[:, db:db + 1])

                if _rep == 0 or "2" in phases:
                    # ---- Phase A2: V projection (s-major, ones column per head) ----
                    with (
                        tc.tile_pool(name="wvp", bufs=1) as wpool,
                        tc.tile_pool(name="xvp", bufs=3) as xpool,
                        tc.tile_pool(name="psv", bufs=2, space="PSUM") as pspool,
                    ):
                        wv_sb = wpool.tile([128, NB_K, DL], F32R, name="wv", tag="wv")
                        nc.sync.dma_start(
                            wv_sb[:], wv.ap().rearrange("(kt p) n -> p kt n", p=128))
                        xvT_r = xvT.ap().rearrange("(kt p) n -> p kt n", p=128)
                        for m in range(NB_S):
                            xt = xpool.tile([128, NB_K, 128], F32R, name="xv", tag="xv")
                            nc.sync.dma_start(xt[:], xvT_r[:, :, m * 128:(m + 1) * 128])
                            ps = pspool.tile([128, 512], F32, name="pv", tag="pv")
                            for k in range(NB_K):
                                nc.tensor.matmul(ps[:], xt[:, k, :], wv_sb[:, k, :],
                                                 start=(k == 0), stop=False)
                            nc.tensor.matmul(ps[:], ones_sb[0:1, :], bv_sb[0:1, :],
                                             start=False, stop=True)
                            nc.vector.memset(VT[m][:, :, DK:DK + 1].bitcast(F32), 1.0)
                            nc.vector.tensor_copy(
                                VT[m][:, :, 0:DK],
                                ps[:].rearrange("p (h c) -> p h c", c=DK))

                if _rep == 0 or "B" in phases:
                    # ---- Phase B: attention per head-pair / q-block ----
                    with (
                        tc.tile_pool(name="pt", bufs=4) as ptpool,
                        tc.tile_pool(name="nrm", bufs=2) as npool,
                        tc.tile_pool(name="psS", bufs=2, space="PSUM") as spool,
                        tc.tile_pool(name="psO", bufs=1, space="PSUM") as opool,
                        tc.tile_pool(name="psB", bufs=1, space="PSUM") as bpool,
                    ):
                        for hp in range(HL // 2):
                            pair = (2 * hp, 2 * hp + 1)
                            for qb in range(NQ):
                                kbmax = 4 * (qb + 1)
                                pso = {h: opool.tile([128, 512], F32, name=f"o{h % 2}", tag=f"o{h % 2}")
                                       for h in pair}
                                prev = None

                                def emit_av(entry, pso=pso, qb=qb, kbmax=kbmax):
                                    kb, cur = entry
                                    for h, (pt, minq) in cur.items():
                                        nc.tensor.matmul(
                                            pso[h][0:DK + 1, minq:512],
                                            VT[kb][:, h, :],
                                            pt[:, minq:512],
                                            start=(kb == 0), stop=(kb == kbmax - 1),
                                            skip_group_check=True)

                                for kb in range(kbmax):
                                    di = kb - 4 * qb  # >= 0 on diagonal blocks
                                    minq = 0 if di < 0 else (128 * di if di < 3 else 256)
                                    cur = {}
                                    scores = []
                                    for h in pair:
                                        db, base = h // 2, (h % 2) * 64
                                        pss = spool.tile([128, 512], F32,
                                                         name=f"s{h % 2}", tag=f"s{h % 2}")
                                        scores.append((h, pss))
                                        nc.tensor.matmul(
                                            pss[:, minq:512],
                                            KT[db][base:base + 64,
                                                   kb * 128:(kb + 1) * 128],
                                            QT[db][base:base + 64,
                                                   qb * 512 + minq:(qb + 1) * 512],
                                            start=True, stop=(di < 0),
                                            skip_group_check=True)
                                    for h, pss in scores:
                                        if di >= 0:
                                            if di == 3:
                                                nc.tensor.matmul(
                                                    pss[:, 256:384], id_sb[:], rect_sb[:],
                                                    start=False, stop=False,
                                                    skip_group_check=True)
                                            nc.tensor.matmul(
                                                pss[:, di * 128:(di + 1) * 128],
                                                id_sb[:], tri_sb[:],
                                                start=False, stop=True,
                                                skip_group_check=True)
                                        pt = ptpool.tile([128, 512], F32R,
                                                         name=f"p{h % 2}", tag=f"p{h % 2}")
                                        nc.scalar.activation(pt[:, minq:512],
                                                             pss[:, minq:512], Exp)
                                        cur[h] = (pt, minq)
                                    if prev is not None:
                                        emit_av(prev)
                                    prev = (kb, cur)
                                emit_av(prev)

                                for h in pair:
                                    db, base = h // 2, (h % 2) * 64
                                    recip = npool.tile([1, 512], F32R, name="rc", tag="rc")
                                    nc.vector.reciprocal(recip[:], pso[h][DK:DK + 1, :])
                                    psb = bpool.tile([128, 512], F32, name=f"b{h % 2}", tag=f"b{h % 2}")
                                    nc.tensor.matmul(psb[0:DK, :], ones_sb[0:1, 0:DK],
                                                     recip[:], start=True, stop=True,
                                                     skip_group_check=True)
                                    rb = npool.tile([64, 512], F32, name="rb", tag="rb")
                                    nc.vector.tensor_copy(rb[:], psb[0:DK, :])
                                    nc.vector.tensor_mul(
                                        OT[db][base:base + 64, qb * 512:(qb + 1) * 512],
                                        pso[h][0:DK, :], rb[:])

                if _rep == 0 or "C" in phases:
                    # ---- Phase C: output projection ----
                    with (
                        tc.tile_pool(name="wop", bufs=1) as wpool,
                        tc.tile_pool(name="osb", bufs=4) as outpool,
                        tc.tile_pool(name="psC", bufs=4, space="PSUM") as cpool2,
                    ):
                        wo_sb = wpool.tile([128, NB_DB, D], F32R, name="wo", tag="wo")
                        nc.sync.dma_start(
                            wo_sb[:], wo.ap().rearrange("(kt p) n -> p kt n", p=128))
                        for m in range(NB_S):
                            for n2 in range(2):
                                ps = cpool2.tile([128, 512], F32, name="c", tag="c")
                                for db in range(NB_DB):
                                    nc.tensor.matmul(
                                        ps[:],
                                        OT[db][:, m * 128:(m + 1) * 128],
                                        wo_sb[:, db, n2 * 512:(n2 + 1) * 512],
                                        start=(db == 0), stop=False)
                                nc.tensor.matmul(ps[:], ones_sb[0:1, :],
                                                 bo_sb[0:1, n2 * 512:(n2 + 1) * 512],
                                                 start=False, stop=True)
                                ot = outpool.tile([128, 512], F32, name="ob", tag="ob")
                                nc.vector.tensor_copy(ot[:], ps[:])
                                nc.sync.dma_start(
                                    out_d[m * 128:(m + 1) * 128,
                                          n2 * 512:(n2 + 1) * 512], ot[:])


    nc.finalize()
    return nc


def _make_in_maps(query, value, key, Wq, bq, Wk, bk, Wv, bv, Wo, bo):
    import ml_dtypes

    f32 = np.float32
    query = np.asarray(query, f32)
    value = np.asarray(value, f32)
    key = np.asarray(key, f32)
    Wq = np.asarray(Wq, f32); bq = np.asarray(bq, f32)
    Wk = np.asarray(Wk, f32); bk = np.asarray(bk, f32)
    Wv = np.asarray(Wv, f32); bv = np.asarray(bv, f32)
    Wo = np.asarray(Wo, f32); bo = np.asarray(bo, f32)

    p = np.arange(128)[:, None]
    j = np.arange(128)[None, :]
    tri = np.where(p > j, MASK_VAL, 0.0).astype(ml_dtypes.bfloat16)
    rect = np.full((128, 128), MASK_VAL, ml_dtypes.bfloat16)
    identb = np.eye(128, dtype=ml_dtypes.bfloat16)
    onesr = np.ones((1, 128), f32)

    in_maps = []
    for b in range(4):
        for g in range(2):
            sl = slice(g * DL, (g + 1) * DL)
            m = {
                "xqT": np.ascontiguousarray(query[b].T),
                "xkT": np.ascontiguousarray(key[b].T),
                "xvT": np.ascontiguousarray(value[b].T),
                "wq": np.ascontiguousarray(Wq[:, sl] / 8.0),
                "wk": np.ascontiguousarray(Wk[:, sl]),
                "wv": np.ascontiguousarray(Wv[:, sl]),
                "wo": np.ascontiguousarray(Wo[sl, :]),
                "bqs": np.ascontiguousarray((bq[sl] / 8.0).reshape(NB_DB, 128).T),
                "bks": np.ascontiguousarray(bk[sl].reshape(NB_DB, 128).T),
                "bvrow": np.ascontiguousarray(bv[sl].reshape(1, DL)),
                "borow": (bo if g == 0 else np.zeros_like(bo)).reshape(1, D).copy(),
                "tri": tri, "rect": rect, "identb": identb, "onesr": onesr,
            }
            in_maps.append(m)
    return in_maps


def kernel_with_info(inputs, trace=False, reps=1):
    from concourse.bass_utils import run_bass_kernel_spmd

    if reps not in _NC:
        _NC[reps] = _build_nc(reps)

    in_maps = _make_in_maps(**inputs)
    res = run_bass_kernel_spmd(_NC[reps], in_maps, core_ids=list(range(8)),
                               trace=trace)
    out = np.empty((4, S, D), np.float32)
    for b in range(4):
        out[b] = res.results[2 * b]["out"] + res.results[2 * b + 1]["out"]
    return out, res


def kernel(**inputs):
    out, _ = kernel_with_info(inputs)
    return out



# revision 3
# speedup vs baseline: 1.9079x; 1.9079x over previous
"""Multi-head attention (B=4, S=2048, D=1024, H=16, causal) on 8 trn2 cores.

Sharding: core = b*2 + g  (b = batch 0..3, g = head-group 0..1, 8 heads each).
All matmul operands are bf16 (enables Fast Weight Load on LDWEIGHTS and halves
input DMA); PSUM accumulation stays fp32 so precision loss is only on operand
rounding (~0.3% worst-case, well inside the 2e-2 gate).

Per core:
  A: Q^T/K^T projections (d-major: QT/KT[db] = [128 d, 2048 s], db = head pair)
     and V projection (s-major VT[kb] = [128 s, 8 heads, 64+1] with an appended
     ones column per head -- the AV matmul then computes softmax denominators
     in PSUM row 64 for free).
  B: per head-pair / q-block of 512: transposed scores S^T = K_h Q_h^T per
     128x512 key block, both heads of the pair row-tiled into disjoint PE row
     groups (concurrent), accumulated into one [128,2,512] PSUM pair tile;
     causal handling via block skipping + additive -60 triangle mask matmul;
     one Exp ACTIVATE per kb covers both heads; O^T accumulated over key
     blocks in a [128,2,512] PSUM pair; normalization = approx-reciprocal of
     the denominator row, PE broadcast, DVE multiply into OT (bf16).
  C: output projection out = OT^T @ Wo + bo, staged through ScalarE copies.
Host: input transpose/bf16-cast + shard, and the g-pair partial-sum
(row-parallel Wo all-reduce) at gather time.
"""

import numpy as np

S = 2048
D = 1024
DL = 512          # local head dims per core (8 heads x 64)
HL = 8            # local heads
DK = 64
NBK = D // 128    # contraction tiles for projections
NDB = DL // 128   # d-out blocks (head pairs)
NQ = S // 512     # q blocks
NBS = S // 128    # s tiles / key blocks
MASK_VAL = -60.0

_NC = {}


def _build_nc():
    import concourse.bass as bass
    import concourse.mybir as mybir
    import concourse.tile as tile
    from concourse import bacc

    F32 = mybir.dt.float32
    F32R = mybir.dt.float32r
    BF16 = mybir.dt.bfloat16
    Exp = mybir.ActivationFunctionType.Exp

    nc = bacc.Bacc(None)

    xq = nc.dram_tensor("xq", [128, NBK, S], BF16, kind="ExternalInput")
    xk = nc.dram_tensor("xk", [128, NBK, S], BF16, kind="ExternalInput")
    xv = nc.dram_tensor("xv", [128, NBK, S], BF16, kind="ExternalInput")
    wq = nc.dram_tensor("wq", [128, NBK, DL], BF16, kind="ExternalInput")
    wk = nc.dram_tensor("wk", [128, NBK, DL], BF16, kind="ExternalInput")
    wv = nc.dram_tensor("wv", [128, NBK, DL], BF16, kind="ExternalInput")
    wo = nc.dram_tensor("wo", [128, NDB, D], BF16, kind="ExternalInput")
    bqs = nc.dram_tensor("bqs", [128, NDB], F32, kind="ExternalInput")
    bks = nc.dram_tensor("bks", [128, NDB], F32, kind="ExternalInput")
    bvr = nc.dram_tensor("bvr", [1, DL], BF16, kind="ExternalInput")
    bor = nc.dram_tensor("bor", [1, D], BF16, kind="ExternalInput")
    tri = nc.dram_tensor("tri", [128, 128], BF16, kind="ExternalInput")
    idn = nc.dram_tensor("idn", [128, 128], BF16, kind="ExternalInput")
    onesb = nc.dram_tensor("onesb", [1, 128], BF16, kind="ExternalInput")
    onesr = nc.dram_tensor("onesr", [1, DK], F32R, kind="ExternalInput")
    out_d = nc.dram_tensor("out", [S, D], F32, kind="ExternalOutput")

    with tile.TileContext(nc) as tc, nc.allow_low_precision(
            reason="bf16 matmul operands are intended"):
        with (
            tc.tile_pool(name="const", bufs=1) as cpool,
            tc.tile_pool(name="res", bufs=1) as rpool,
            tc.tile_pool(name="xt", bufs=3) as xpool,
            tc.tile_pool(name="pt", bufs=3) as ptpool,
            tc.tile_pool(name="rc", bufs=2) as rcpool,
            tc.tile_pool(name="rb", bufs=2) as rbpool,
            tc.tile_pool(name="ot", bufs=3) as otpool,
            tc.tile_pool(name="mp", bufs=2, space="PSUM") as mpool,
            tc.tile_pool(name="sp", bufs=2, space="PSUM") as spool,
            tc.tile_pool(name="op", bufs=1, space="PSUM") as opool,
        ):
            tri_sb = cpool.tile([128, 128], BF16, name="tri", tag="tri")
            id_sb = cpool.tile([128, 128], BF16, name="idn", tag="idn")
            onesb_sb = cpool.tile([1, 128], BF16, name="onesb", tag="onesb")
            onesr_sb = cpool.tile([1, DK], F32R, name="onesr", tag="onesr")
            bqs_sb = cpool.tile([128, NDB], F32, name="bqs", tag="bqs")
            bks_sb = cpool.tile([128, NDB], F32, name="bks", tag="bks")
            bv_sb = cpool.tile([1, DL], BF16, name="bv", tag="bv")
            bo_sb = cpool.tile([1, D], BF16, name="bo", tag="bo")
            for t, dt_ in [(tri_sb, tri), (id_sb, idn), (onesb_sb, onesb),
                           (onesr_sb, onesr), (bqs_sb, bqs), (bks_sb, bks),
                           (bv_sb, bvr), (bo_sb, bor)]:
                nc.sync.dma_start(t[:], dt_[:])

            wq_sb = rpool.tile([128, NBK, DL], BF16, name="wq", tag="wq")
            wk_sb = rpool.tile([128, NBK, DL], BF16, name="wk", tag="wk")
            wv_sb = rpool.tile([128, NBK, DL], BF16, name="wv", tag="wv")
            wo_sb = rpool.tile([128, NDB, D], BF16, name="wo", tag="wo")
            for t, dt_ in [(wq_sb, wq), (wk_sb, wk), (wv_sb, wv), (wo_sb, wo)]:
                nc.sync.dma_start(t[:], dt_[:])

            QT = [rpool.tile([128, S], BF16, name=f"QT{i}", tag=f"QT{i}") for i in range(NDB)]
            KT = [rpool.tile([128, S], BF16, name=f"KT{i}", tag=f"KT{i}") for i in range(NDB)]
            VT = [rpool.tile([128, HL, DK + 1], BF16, name=f"VT{i}", tag=f"VT{i}")
                  for i in range(NBS)]
            OT = [rpool.tile([128, S], BF16, name=f"OT{i}", tag=f"OT{i}") for i in range(NDB)]

            # ---- Phase A: projections, one 512-wide sequence chunk at a time ----
            for s in range(NQ):
                sl = slice(s * 512, (s + 1) * 512)
                for xd, w_sb_, b_sb, dst in ((xq, wq_sb, bqs_sb, QT),
                                             (xk, wk_sb, bks_sb, KT)):
                    xt = xpool.tile([128, NBK, 512], BF16, name="xt", tag="xt")
                    nc.sync.dma_start(xt[:], xd[:, :, sl])
                    for db in range(NDB):
                        ps = mpool.tile([128, 512], F32, name="mp", tag="mp")
                        for k in range(NBK):
                            nc.tensor.matmul(
                                ps[:], w_sb_[:, k, db * 128:(db + 1) * 128],
                                xt[:, k, :], start=(k == 0), stop=(k == NBK - 1))
                        nc.vector.tensor_scalar_add(
                            dst[db][:, sl], ps[:], b_sb[:, db:db + 1])
                xtv = xpool.tile([128, NBK, 512], BF16, name="xt", tag="xt")
                nc.sync.dma_start(xtv[:], xv[:, :, sl])
                for mi in range(4):
                    m = 4 * s + mi
                    ps = mpool.tile([128, 512], F32, name="mp", tag="mp")
                    for k in range(NBK):
                        nc.tensor.matmul(
                            ps[:], xtv[:, k, mi * 128:(mi + 1) * 128],
                            wv_sb[:, k, :], start=(k == 0), stop=False)
                    nc.tensor.matmul(ps[:], onesb_sb[0:1, :], bv_sb[0:1, :],
                                     start=False, stop=True)
                    nc.vector.memset(VT[m][:, :, DK:DK + 1], 1.0)
                    nc.vector.tensor_copy(
                        VT[m][:, :, 0:DK],
                        ps[:].rearrange("p (h c) -> p h c", c=DK))

            # ---- Phase B: attention per head-pair / q-block ----
            for hp in range(NDB):
                for qb in range(NQ):
                    kbmax = 4 * (qb + 1)
                    qsl = slice(qb * 512, (qb + 1) * 512)
                    pso = opool.tile([128, 2, 512], F32, name="op", tag="op")
                    prev = None

                    def emit_av(entry, pso=pso, kbmax=kbmax, hp=hp):
                        kb, pt_, minq = entry
                        for h2 in range(2):
                            nc.tensor.matmul(
                                pso[0:DK + 1, h2, minq:512],
                                VT[kb][:, hp * 2 + h2, :],
                                pt_[:, h2, minq:512],
                                start=(kb == 0), stop=(kb == kbmax - 1),
                                skip_group_check=True)

                    for kb in range(kbmax):
                        di = kb - 4 * qb
                        minq = 128 * di if di > 0 else 0
                        pss = spool.tile([128, 2, 512], F32, name="sp", tag="sp")
                        for h2 in range(2):
                            base = h2 * DK
                            nc.tensor.matmul(
                                pss[:, h2, minq:512],
                                KT[hp][base:base + DK, kb * 128:(kb + 1) * 128],
                                QT[hp][base:base + DK, qb * 512 + minq:(qb + 1) * 512],
                                start=True, stop=(di < 0),
                                skip_group_check=True)
                        if di >= 0:
                            for h2 in range(2):
                                nc.tensor.matmul(
                                    pss[:, h2, minq:minq + 128], id_sb[:], tri_sb[:],
                                    start=False, stop=True,
                                    skip_group_check=True)
                        pt_ = ptpool.tile([128, 2, 512], BF16, name="pt", tag="pt")
                        nc.scalar.activation(pt_[:, :, minq:512],
                                             pss[:, :, minq:512], Exp)
                        if prev is not None:
                            emit_av(prev)
                        prev = (kb, pt_, minq)
                    emit_av(prev)

                    # normalization: OT = pso[0:64] * (1 / denom-row)
                    den = rcpool.tile([1, 2, 512], F32R, name="rc", tag="rc")
                    nc.vector.tensor_copy(den[:], pso[DK:DK + 1, :, :])
                    rb = rbpool.tile([DK, 2, 512], F32, name="rb", tag="rb")
                    for h2 in range(2):
                        psb = mpool.tile([DK, 512], F32, name="mp", tag="mp")
                        nc.tensor.matmul(psb[:], onesr_sb[0:1, :],
                                         den[0:1, h2, :],
                                         start=True, stop=True,
                                         skip_group_check=True)
                        nc.vector.reciprocal_approx_fast(
                            out=rb[:, h2, :], in_=psb[:])
                    for h2 in range(2):
                        nc.vector.tensor_mul(
                            OT[hp][h2 * DK:(h2 + 1) * DK, qsl],
                            pso[0:DK, h2, :], rb[:, h2, :])

            # ---- Phase C: output projection ----
            for m in range(NBS):
                msl = slice(m * 128, (m + 1) * 128)
                for n2 in range(2):
                    nsl = slice(n2 * 512, (n2 + 1) * 512)
                    ps = mpool.tile([128, 512], F32, name="mp", tag="mp")
                    for db in range(NDB):
                        nc.tensor.matmul(
                            ps[:], OT[db][:, msl], wo_sb[:, db, nsl],
                            start=(db == 0), stop=False)
                    nc.tensor.matmul(ps[:], onesb_sb[0:1, :], bo_sb[0:1, nsl],
                                     start=False, stop=True)
                    ot = otpool.tile([128, 512], F32, name="ob", tag="ob")
                    nc.scalar.copy(ot[:], ps[:])
                    nc.sync.dma_start(out_d[msl, nsl], ot[:])

    nc.finalize()
    return nc


def _to_pkt(a2d, nt):
    """[nt*128, N] -> [128, nt, N] (partition-major tiling of the first dim)."""
    n = a2d.shape[1]
    return np.ascontiguousarray(
        a2d.reshape(nt, 128, n).transpose(1, 0, 2))


def _make_in_maps(query, value, key, Wq, bq, Wk, bk, Wv, bv, Wo, bo):
    import ml_dtypes

    f32 = np.float32
    bf16 = ml_dtypes.bfloat16
    query = np.asarray(query, f32)
    value = np.asarray(value, f32)
    key = np.asarray(key, f32)
    Wq = np.asarray(Wq, f32); bq = np.asarray(bq, f32)
    Wk = np.asarray(Wk, f32); bk = np.asarray(bk, f32)
    Wv = np.asarray(Wv, f32); bv = np.asarray(bv, f32)
    Wo = np.asarray(Wo, f32); bo = np.asarray(bo, f32)

    p = np.arange(128)[:, None]
    j = np.arange(128)[None, :]
    tri = np.where(p > j, MASK_VAL, 0.0).astype(bf16)
    idn = np.eye(128, dtype=bf16)
    onesb = np.ones((1, 128), bf16)
    onesr = np.ones((1, DK), f32)

    xT = {}
    for nm, x in (("q", query), ("k", key), ("v", value)):
        xT[nm] = [_to_pkt(x[b].T.astype(bf16), NBK) for b in range(4)]

    in_maps = []
    for b in range(4):
        for g in range(2):
            sl = slice(g * DL, (g + 1) * DL)
            m = {
                "xq": xT["q"][b],
                "xk": xT["k"][b],
                "xv": xT["v"][b],
                "wq": _to_pkt((Wq[:, sl] / 8.0).astype(bf16), NBK),
                "wk": _to_pkt(Wk[:, sl].astype(bf16), NBK),
                "wv": _to_pkt(Wv[:, sl].astype(bf16), NBK),
                "wo": _to_pkt(Wo[sl, :].astype(bf16), NDB),
                "bqs": np.ascontiguousarray((bq[sl] / 8.0).reshape(NDB, 128).T),
                "bks": np.ascontiguousarray(bk[sl].reshape(NDB, 128).T),
                "bvr": bv[sl].reshape(1, DL).astype(bf16),
                "bor": (bo if g == 0 else np.zeros_like(bo)).reshape(1, D).astype(bf16),
                "tri": tri, "idn": idn, "onesb": onesb, "onesr": onesr,
            }
            in_maps.append(m)
    return in_maps


def kernel_with_info(inputs, trace=False):
    from concourse.bass_utils import run_bass_kernel_spmd

    if "nc" not in _NC:
        _NC["nc"] = _build_nc()

    in_maps = _make_in_maps(**inputs)
    res = run_bass_kernel_spmd(_NC["nc"], in_maps, core_ids=list(range(8)),
                               trace=trace)
    out = np.empty((4, S, D), np.float32)
    for b in range(4):
        out[b] = res.results[2 * b]["out"] + res.results[2 * b + 1]["out"]
    return out, res


def kernel(**inputs):
    out, _ = kernel_with_info(inputs)
    return out
